# revision 18
# baseline (speedup 1.0000x reference)
"""NormLinearAttention Trainium2 kernel — 8-core sequence-parallel Bass/Tile.

Math (reference):
    q = k = elu(heads(x @ Wqk + bqk));  v = heads(silu(x @ Wv + bv))
    u = silu(x @ Wu + bu)
    kv[b,h] = k^T v  (contract over sequence);  att = q @ kv
    y = (u * layernorm(att)) @ Wo + bo

Sharding: each of 8 cores owns 512 tokens of each batch (2048 tokens total).
Per-core partial kv[b,h,d,e] is AllReduce-summed across cores (4 MB), fully
overlapped with the u-projection; everything else is local.

Layout plan (per core):
  host feeds xT [E, 2048tok] bf16 so projections run without transposing x.
   - q,v computed in natural layout [tok, j] (needed by the kv einsum, which
     contracts over tokens = partition dim).
   - q tiles are PE-transposed on the fly to qT [j, tok] and spilled to DRAM
     for the attention einsum.
   - u and att are computed TRANSPOSED ([ch, tok]) so layernorm stats run as
     ones-vector matmuls over the partition dim, the per-channel ln_g/ln_b
     become cheap per-partition tensor_scalar fusions, and no z-transposes are
     needed before the final projection.
All matmuls bf16 (fp32 PSUM accumulate); elementwise/LN math fp32.
The host detects zero biases / identity layernorm affine and builds the
kernel without the corresponding ops (the reference generator uses zeros).
"""

import sys

if "/opt/trn_rl_repo" not in sys.path:
    sys.path.insert(0, "/opt/trn_rl_repo")

import numpy as np
import ml_dtypes

B, N, E = 4, 4096, 2048
H_DIM, HEADS, DH = 2048, 16, 128
N_CORES = 8
NL = N // N_CORES          # 512 tokens per (core, batch)
TL = B * NL                # 2048 local tokens per core
ET = E // 128              # 16 contraction tiles
JT = H_DIM // 128          # 16 hidden tiles
NJQ = 4                    # process hidden dim in 4 quarters of 512
TOK_B = NL // 128          # 4 token tiles per batch
LN_EPS = 1e-5

_BUILT = {}


def _build(flags, debug=False):
    import concourse.bacc as bacc
    import concourse.mybir as mybir
    import concourse.tile as tile
    from concourse.masks import make_identity

    has_bqv, has_bu, has_bo, has_affine = flags
    f32 = mybir.dt.float32
    bf16 = mybir.dt.bfloat16

    nc = bacc.Bacc("TRN2", target_bir_lowering=False, debug=False,
                   num_devices=N_CORES)

    t = {}
    t["xT"] = nc.dram_tensor("xT", [E, TL], bf16, kind="ExternalInput").ap()
    t["wqk"] = nc.dram_tensor("wqk", [E, H_DIM], bf16, kind="ExternalInput").ap()
    t["wv"] = nc.dram_tensor("wv", [E, H_DIM], bf16, kind="ExternalInput").ap()
    t["wu"] = nc.dram_tensor("wu", [E, H_DIM], bf16, kind="ExternalInput").ap()
    t["wo"] = nc.dram_tensor("wo", [H_DIM, E], bf16, kind="ExternalInput").ap()
    if has_bqv:
        t["bqk_r"] = nc.dram_tensor("bqk_r", [1, H_DIM], bf16,
                                    kind="ExternalInput").ap()
        t["bv_r"] = nc.dram_tensor("bv_r", [1, H_DIM], bf16,
                                   kind="ExternalInput").ap()
    if has_bo:
        t["bo_r"] = nc.dram_tensor("bo_r", [1, E], bf16,
                                   kind="ExternalInput").ap()
    if has_bu:
        t["bu_c"] = nc.dram_tensor("bu_c", [128, JT], f32,
                                   kind="ExternalInput").ap()
    if has_affine:
        t["g_c"] = nc.dram_tensor("g_c", [128, JT], f32,
                                  kind="ExternalInput").ap()
        t["b_c"] = nc.dram_tensor("b_c", [128, JT], f32,
                                  kind="ExternalInput").ap()
    t["y"] = nc.dram_tensor("y", [TL, E], f32, kind="ExternalOutput").ap()

    dbg = None
    if debug:
        dbg = {
            "qT": nc.dram_tensor("dbg_qT", [TL, H_DIM], bf16,
                                 kind="ExternalOutput").ap(),
            "uT": nc.dram_tensor("dbg_uT", [H_DIM, TL], bf16,
                                 kind="ExternalOutput").ap(),
            "kvin": nc.dram_tensor("dbg_kvin", [B * HEADS * 128, DH], f32,
                                   kind="ExternalOutput").ap(),
            "kvout": nc.dram_tensor("dbg_kvout", [B * HEADS * 128, DH], f32,
                                    kind="ExternalOutput").ap(),
            "att": nc.dram_tensor("dbg_att", [B * 128, HEADS * NL], bf16,
                                  kind="ExternalOutput").ap(),
            "zT": nc.dram_tensor("dbg_zT", [B * 128, JT * NL], bf16,
                                 kind="ExternalOutput").ap(),
        }
    with tile.TileContext(nc) as tc:
        _body(nc, tc, tile, mybir, make_identity, f32, bf16, t, flags, dbg)
    nc.compile()
    return nc


def _body(nc, tc, tile, mybir, make_identity, f32, bf16, t, flags, dbg=None):
    Act = mybir.ActivationFunctionType
    Alu = mybir.AluOpType
    has_bqv, has_bu, has_bo, has_affine = flags

    with (
        tc.tile_pool(name="consts", bufs=1) as consts,
        tc.tile_pool(name="dram", bufs=1, space="DRAM") as dram,
    ):
        ones_col = consts.tile([128, 1], bf16)
        nc.vector.memset(ones_col, 1.0)
        ones_row = consts.tile([1, 128], f32)
        nc.vector.memset(ones_row, 1.0)
        eps_sb = consts.tile([1, 1], f32)
        nc.vector.memset(eps_sb, LN_EPS)
        if has_bqv or has_bo:
            ones_bf = consts.tile([1, 128], bf16)
            nc.vector.memset(ones_bf, 1.0)
        if has_bqv:
            bqk_sb = consts.tile([1, H_DIM], bf16)
            nc.sync.dma_start(bqk_sb[:], t["bqk_r"][:])
            bv_sb = consts.tile([1, H_DIM], bf16)
            nc.sync.dma_start(bv_sb[:], t["bv_r"][:])
        if has_bo:
            bo_sb = consts.tile([1, E], bf16)
            nc.sync.dma_start(bo_sb[:], t["bo_r"][:])
        if has_bu:
            bu_sb = consts.tile([128, JT], f32)
            nc.sync.dma_start(bu_sb[:], t["bu_c"][:])
        if has_affine:
            g_sb = consts.tile([128, JT], f32)
            nc.sync.dma_start(g_sb[:], t["g_c"][:])
            b_sb = consts.tile([128, JT], f32)
            nc.sync.dma_start(b_sb[:], t["b_c"][:])

        q_dram = dram.tile([TL, H_DIM], bf16)
        uT_dram = dram.tile([H_DIM, TL], bf16)
        cc_in = dram.tile([B * HEADS * 128, DH], f32)
        cc_out = dram.tile([B * HEADS * 128, DH], f32, addr_space="Shared")

        with tc.tile_pool(name="xt_pool", bufs=1) as xt_pool:
            xt = xt_pool.tile([128, ET, TL], bf16)   # 8 MB resident ph1-2
            for tt in range(ET):
                nc.sync.dma_start(
                    xt[:, tt], t["xT"].rearrange("(t p) n -> t p n", p=128)[tt])

            # w2 spans phases 1+2 with SBUF addresses disjoint from w1/st1,
            # so the wu prefetch overlaps phase 1 (no phase-boundary stall)
            w2_ctx = tc.tile_pool(name="w2", bufs=1)
            w2 = w2_ctx.__enter__()

            # ---------------- phase 1: q/v projections + partial kv --------
            with (
                tc.tile_pool(name="w1", bufs=1) as w1,
                tc.tile_pool(name="st1", bufs=1) as st1,
                tc.tile_pool(name="ps_proj", bufs=1, space="PSUM") as psp,
                tc.tile_pool(name="ps_kv", bufs=1, space="PSUM") as pskv,
            ):
                for jq in range(NJQ):
                    wq_sb = w1.tile([128, ET, 512], bf16, tag="wq", bufs=2)
                    nc.sync.dma_start(
                        wq_sb[:],
                        t["wqk"][:, jq * 512:(jq + 1) * 512]
                        .rearrange("(t p) j -> p t j", p=128))
                    wv_sb = w1.tile([128, ET, 512], bf16, tag="wv", bufs=2)
                    nc.sync.dma_start(
                        wv_sb[:],
                        t["wv"][:, jq * 512:(jq + 1) * 512]
                        .rearrange("(t p) j -> p t j", p=128))
                    for b in range(B):
                        q_tiles, v_tiles = [], []
                        for tk in range(TOK_B):
                            tok0 = b * NL + tk * 128
                            q_ps = psp.tile([128, 512], f32, tag="qps", bufs=3)
                            v_ps = psp.tile([128, 512], f32, tag="vps", bufs=3)
                            for tt in range(ET):
                                lhs = xt[:, tt, tok0:tok0 + 128]
                                nc.tensor.matmul(q_ps[:], lhs, wq_sb[:, tt],
                                                 start=(tt == 0), stop=False)
                                nc.tensor.matmul(
                                    v_ps[:], lhs, wv_sb[:, tt],
                                    start=(tt == 0),
                                    stop=(not has_bqv and tt == ET - 1))
                            if has_bqv:
                                nc.tensor.matmul(
                                    q_ps[:], ones_bf[:],
                                    bqk_sb[:, jq * 512:(jq + 1) * 512],
                                    start=False, stop=True)
                                nc.tensor.matmul(
                                    v_ps[:], ones_bf[:],
                                    bv_sb[:, jq * 512:(jq + 1) * 512],
                                    start=False, stop=True)

                            # elu(q) = (max(q,0) - 1) + exp(min(q, 0))
                            tmin = st1.tile([128, 512], f32, tag="tmin",
                                            bufs=2)
                            nc.vector.tensor_scalar_min(tmin[:], q_ps[:], 0.0)
                            texp = st1.tile([128, 512], f32, tag="texp",
                                            bufs=2)
                            nc.scalar.activation(texp[:], tmin[:], Act.Exp)
                            trelu = st1.tile([128, 512], f32, tag="trelu",
                                             bufs=2)
                            nc.vector.tensor_scalar(trelu[:], q_ps[:], 0.0,
                                                    -1.0, Alu.max, Alu.add)
                            q_bf = st1.tile([128, 512], bf16, tag="qbf",
                                            bufs=5)
                            nc.vector.tensor_add(q_bf[:], trelu[:], texp[:])
                            v_bf = st1.tile([128, 512], bf16, tag="vbf",
                                            bufs=5)
                            nc.scalar.activation(v_bf[:], v_ps[:], Act.Silu)
                            q_tiles.append(q_bf)
                            v_tiles.append(v_bf)

                            # spill q (natural layout) for phase 3
                            nc.sync.dma_start(
                                q_dram[tok0:tok0 + 128,
                                       jq * 512:(jq + 1) * 512],
                                q_bf[:])

                        # per-head contiguous kv accumulation: each head owns
                        # a whole PSUM bank (start=True clears the full bank,
                        # so accumulation groups must not share banks)
                        kv_sb = st1.tile([128, 4, DH], f32, tag="kvsb",
                                         bufs=2)
                        for h in range(4):
                            kv_ps = pskv.tile([128, DH], f32, tag="kv",
                                              bufs=2)
                            for tk in range(TOK_B):
                                nc.tensor.matmul(
                                    kv_ps[:],
                                    q_tiles[tk][:, h * 128:(h + 1) * 128],
                                    v_tiles[tk][:, h * 128:(h + 1) * 128],
                                    start=(tk == 0), stop=(tk == TOK_B - 1))
                            nc.vector.tensor_copy(kv_sb[:, h], kv_ps[:])
                        r0 = (b * HEADS + jq * 4) * 128
                        nc.sync.dma_start(
                            cc_in[r0:r0 + 512, :]
                            .rearrange("(h d) e -> d h e", h=4),
                            kv_sb[:])

            # ---------------- kv AllReduce across the 8 cores --------------
            nc.gpsimd.collective_compute(
                "AllReduce", mybir.AluOpType.add,
                replica_groups=[list(range(N_CORES))],
                ins=[cc_in.opt()], outs=[cc_out.opt()])

            # ---------------- phase 2: uT projection (overlaps AR) ---------
            with (
                tc.tile_pool(name="ps_u", bufs=1, space="PSUM") as psu,
            ):
                for jt in range(JT):
                    wu_sb = w2.tile([128, ET, 128], bf16, tag="wu", bufs=3)
                    nc.sync.dma_start(
                        wu_sb[:],
                        t["wu"][:, jt * 128:(jt + 1) * 128]
                        .rearrange("(t p) j -> p t j", p=128))
                    u_ps = psu.tile([128, 4, 512], f32, tag="ups", bufs=2)
                    for tt in range(ET):
                        for c in range(4):
                            nc.tensor.matmul(
                                u_ps[:, c], wu_sb[:, tt],
                                xt[:, tt, c * 512:(c + 1) * 512],
                                start=(tt == 0), stop=(tt == ET - 1))
                    u_st = w2.tile([128, TL], bf16, tag="ust", bufs=2)
                    ubias = bu_sb[:, jt:jt + 1] if has_bu else 0.0
                    for c in range(4):
                        nc.scalar.activation(
                            u_st[:, c * 512:(c + 1) * 512], u_ps[:, c],
                            Act.Silu, bias=ubias)
                    nc.sync.dma_start(
                        uT_dram[jt * 128:(jt + 1) * 128, :], u_st[:])
            w2_ctx.__exit__(None, None, None)

        if dbg is not None:
            nc.sync.dma_start(dbg["qT"][:], q_dram[:])
            nc.sync.dma_start(dbg["uT"][:], uT_dram[:])
            nc.sync.dma_start(dbg["kvin"][:], cc_in[:])
            nc.sync.dma_start(dbg["kvout"][:], cc_out[:])

        # ------------- phase 3+4: attention, layernorm, output proj --------
        with (
            tc.tile_pool(name="wo_pool", bufs=1) as wo_pool,
            tc.tile_pool(name="st3", bufs=1) as st3,
            tc.tile_pool(name="ps_att", bufs=1, space="PSUM") as psa,
            tc.tile_pool(name="ps_sm", bufs=1, space="PSUM") as pssm,
            tc.tile_pool(name="ps_y", bufs=1, space="PSUM") as psy,
        ):
            wo_sb = wo_pool.tile([128, JT, E], bf16)     # 8 MB resident

            for b in range(B):
                kv_f = st3.tile([128, HEADS, DH], f32, tag="kvf", bufs=1)
                nc.sync.dma_start(
                    kv_f[:],
                    cc_out[b * HEADS * 128:(b + 1) * HEADS * 128, :]
                    .rearrange("(h d) e -> d h e", h=HEADS))
                kv_bf = st3.tile([128, HEADS, DH], bf16, tag="kvbf", bufs=1)
                nc.vector.tensor_copy(kv_bf[:], kv_f[:])
                uT_b = st3.tile([128, JT, NL], bf16, tag="utb", bufs=1)
                nc.sync.dma_start(
                    uT_b[:],
                    uT_dram[:, b * NL:(b + 1) * NL]
                    .rearrange("(jt p) n -> p jt n", p=128))

                att = st3.tile([128, HEADS, NL], bf16, tag="att", bufs=2)
                for h in range(HEADS):
                    qb = st3.tile([128, NL], bf16, tag="qb", bufs=3)
                    nc.sync.dma_start_transpose(
                        qb[:],
                        q_dram[b * NL:(b + 1) * NL, h * 128:(h + 1) * 128])
                    att_ps = psa.tile([128, NL], f32, tag="attps", bufs=2)
                    nc.tensor.matmul(att_ps[:], kv_bf[:, h], qb[:],
                                     start=True, stop=True)
                    nc.scalar.copy(att[:, h], att_ps[:])

                if b == 0:
                    # Wo load deferred here so the kv/qT DMAs above are not
                    # queued behind 8 MB of weight traffic
                    for ct in range(JT):
                        nc.sync.dma_start(
                            wo_sb[:, ct],
                            t["wo"].rearrange("(t p) e -> t p e", p=128)[ct])

                if dbg is not None:
                    nc.sync.dma_start(dbg["att"][b * 128:(b + 1) * 128, :],
                                      att[:])

                # LN stats over channels via ones-matmuls (bf16 operands)
                sum_ps = pssm.tile([1, NL], f32, tag="sum", bufs=1)
                ssq_ps = pssm.tile([1, NL], f32, tag="ssq", bufs=1)
                for h in range(HEADS):
                    nc.tensor.matmul(sum_ps[:], ones_col[:], att[:, h],
                                     start=(h == 0), stop=(h == HEADS - 1))
                for h in range(HEADS):
                    sq = st3.tile([128, NL], bf16, tag="sq", bufs=2)
                    nc.vector.tensor_mul(sq[:], att[:, h], att[:, h])
                    nc.tensor.matmul(ssq_ps[:], ones_col[:], sq[:],
                                     start=(h == 0), stop=(h == HEADS - 1))
                mean = st3.tile([1, NL], f32, tag="mean", bufs=1)
                nc.vector.tensor_scalar_mul(mean[:], sum_ps[:], 1.0 / H_DIM)
                msq = st3.tile([1, NL], f32, tag="msq", bufs=1)
                nc.vector.tensor_scalar_mul(msq[:], ssq_ps[:], 1.0 / H_DIM)
                m2 = st3.tile([1, NL], f32, tag="m2", bufs=1)
                nc.vector.tensor_mul(m2[:], mean[:], mean[:])
                var = st3.tile([1, NL], f32, tag="var", bufs=1)
                nc.vector.tensor_sub(var[:], msq[:], m2[:])
                std = st3.tile([1, NL], f32, tag="std", bufs=1)
                nc.scalar.activation(std[:], var[:], Act.Sqrt, bias=eps_sb[:])
                rstd = st3.tile([1, NL], f32, tag="rstd", bufs=1)
                nc.vector.reciprocal(rstd[:], std[:])
                mr = st3.tile([1, NL], f32, tag="mr", bufs=1)
                nc.vector.tensor_mul(mr[:], mean[:], rstd[:])

                rstd_bc = st3.tile([128, NL], f32, tag="rstdbc", bufs=2)
                nc.gpsimd.partition_broadcast(rstd_bc[:], rstd[:])
                mr_bc = st3.tile([128, NL], f32, tag="mrbc", bufs=2)
                nc.gpsimd.partition_broadcast(mr_bc[:], mr[:])

                # zT = ((att * rstd - mean*rstd) [* g + b]) * uT   (bf16)
                zT = st3.tile([128, JT, NL], bf16, tag="zT", bufs=2)
                for jt in range(JT):
                    s1 = st3.tile([128, NL], f32, tag="s1", bufs=2)
                    nc.vector.tensor_mul(s1[:], att[:, jt], rstd_bc[:])
                    s2 = st3.tile([128, NL], f32, tag="s2", bufs=2)
                    nc.vector.tensor_sub(s2[:], s1[:], mr_bc[:])
                    if has_affine:
                        s3 = st3.tile([128, NL], f32, tag="s3", bufs=2)
                        nc.vector.tensor_scalar(
                            s3[:], s2[:], g_sb[:, jt:jt + 1],
                            b_sb[:, jt:jt + 1], Alu.mult, Alu.add)
                    else:
                        s3 = s2
                    nc.vector.tensor_mul(zT[:, jt], s3[:], uT_b[:, jt])

                if dbg is not None:
                    nc.sync.dma_start(dbg["zT"][b * 128:(b + 1) * 128, :],
                                      zT[:])

                # y = zT.T @ Wo + bo
                for tsl in range(TOK_B):
                    ybuf = st3.tile([128, E], f32, tag="ybuf", bufs=2)
                    for ep in range(2):
                        y_ps = psy.tile([128, 2, 512], f32, tag="yps", bufs=2)
                        for ct in range(JT):
                            zslice = zT[:, ct, tsl * 128:(tsl + 1) * 128]
                            for e2 in range(2):
                                e0 = ep * 1024 + e2 * 512
                                nc.tensor.matmul(
                                    y_ps[:, e2], zslice,
                                    wo_sb[:, ct, e0:e0 + 512],
                                    start=(ct == 0),
                                    stop=(not has_bo and ct == JT - 1))
                        if has_bo:
                            for e2 in range(2):
                                e0 = ep * 1024 + e2 * 512
                                nc.tensor.matmul(
                                    y_ps[:, e2], ones_bf[:],
                                    bo_sb[:, e0:e0 + 512],
                                    start=False, stop=True)
                        nc.scalar.copy(
                            ybuf[:, ep * 1024:(ep + 1) * 1024], y_ps[:])
                    nc.sync.dma_start(
                        t["y"][b * NL + tsl * 128: b * NL + (tsl + 1) * 128,
                               :],
                        ybuf[:])


def _get_nc(flags, debug=False):
    key = (flags, debug)
    if key not in _BUILT:
        _BUILT[key] = _build(flags, debug)
    return _BUILT[key]


def make_in_maps(x, Wqk, bqk, Wv, bv, Wu, bu, Wo, bo, ln_g, ln_b):
    bf16 = ml_dtypes.bfloat16
    f32 = np.float32
    x = np.asarray(x)
    flags = (
        bool(np.any(bqk) or np.any(bv)),
        bool(np.any(bu)),
        bool(np.any(bo)),
        bool(np.any(np.asarray(ln_g) != 1.0) or np.any(ln_b)),
    )
    shared = {
        "wqk": np.asarray(Wqk, f32).astype(bf16),
        "wv": np.asarray(Wv, f32).astype(bf16),
        "wu": np.asarray(Wu, f32).astype(bf16),
        "wo": np.asarray(Wo, f32).astype(bf16),
    }
    if flags[0]:
        shared["bqk_r"] = np.asarray(bqk, f32).astype(bf16).reshape(1, H_DIM)
        shared["bv_r"] = np.asarray(bv, f32).astype(bf16).reshape(1, H_DIM)
    if flags[1]:
        shared["bu_c"] = np.ascontiguousarray(
            np.asarray(bu, f32).reshape(JT, 128).T)
    if flags[2]:
        shared["bo_r"] = np.asarray(bo, f32).astype(bf16).reshape(1, E)
    if flags[3]:
        shared["g_c"] = np.ascontiguousarray(
            np.asarray(ln_g, f32).reshape(JT, 128).T)
        shared["b_c"] = np.ascontiguousarray(
            np.asarray(ln_b, f32).reshape(JT, 128).T)
    in_maps = []
    for c in range(N_CORES):
        xc = np.ascontiguousarray(
            x[:, c * NL:(c + 1) * NL, :].reshape(TL, E).T).astype(bf16)
        in_maps.append({"xT": xc, **shared})
    return flags, in_maps


def kernel(x, Wqk, bqk, Wv, bv, Wu, bu, Wo, bo, ln_g, ln_b, **_unused):
    from concourse.bass_utils import run_bass_kernel_spmd

    flags, in_maps = make_in_maps(x, Wqk, bqk, Wv, bv, Wu, bu, Wo, bo,
                                  ln_g, ln_b)
    nc = _get_nc(flags)
    res = run_bass_kernel_spmd(nc, in_maps, core_ids=list(range(N_CORES)))

    y = np.empty((B, N, E), np.float32)
    for c in range(N_CORES):
        y[:, c * NL:(c + 1) * NL, :] = res.results[c]["y"].reshape(B, NL, E)
    return y
